# revision 2
# baseline (speedup 1.0000x reference)
"""Trainium2 Bass kernel for CTAttention (ragged-batch multi-head attention).

Host: pads/scatters ragged rows into [B, NMAX, C], shards batch elements
across 8 NeuronCores (batch b -> core b), derives a per-key 0/1 validity
mask from the additive key mask.

Device (per core, one batch element), engine-balanced design:
  PE    : QKV^T proj (f32r), S = K8^T Q8 per head via fp8e4 DoubleRow
          (head_dim 32 = 32 partitions x 2 sub-tiles, sub-tile 1 zeroed),
          AV with P as stationary and V as moving (output free dim 33),
          O~ transpose via identity matmul (bf16), out proj (bf16).
  ACT   : most exp tiles natively: P = exp(S / A16).
  DVE   : the rest of the exp tiles via bf16 Schraudolph:
          P = bitcast_bf16(i16(round(max(S + B16, 0)))), Q/K pre-scaled by
          sqrt(A16) so S_psum = A16 * S_true; plus psum evacuations and the
          softmax normalization osc = po / rowsum (divide, stride-0 bcast).
  Masking: padded keys are handled by zeroing V rows (and the rowsum ones
          column) with the validity mask during V evacuation - exp needs no
          per-key bias, so any engine can do any tile with immediate bias.
"""

import math
import sys

sys.path.insert(0, "/opt/trn_rl_repo")

import numpy as np

B = 8
NMAX = 1024
C = 256
H = 8
HD = C // H
SCALE = HD ** -0.5

A16 = 128.0 / math.log(2.0)          # bf16 Schraudolph slope
ALPHA = math.sqrt(A16)               # folded into both wq and wk
B16F = 16256.0 - 7.0                 # bf16 exponent bias - error centering

# exp-tile engine assignment per (h, kt): 'a' = ACT native exp,
# 'd' = DVE Schraudolph. DVE also carries the evacuation traffic, so it
# gets fewer tiles, and none on h0 while it drains QKV/V psums.
_EXP_ENG = {}
_DVE_KTS = {h: (1, 3, 5) for h in range(8)}
_DVE_KTS[0] = (3, 5)
_DVE_KTS[2] = (1, 3, 5, 7)
for _h in range(H):
    for _kt in range(8):
        _EXP_ENG[(_h, _kt)] = "d" if _kt in _DVE_KTS[_h] else "a"

_CACHE = {}


def _build_program():
    import concourse.bass as bass
    from concourse import bacc
    import concourse.mybir as mybir
    import concourse.tile as tile

    F32 = mybir.dt.float32
    F32R = mybir.dt.float32r
    BF16 = mybir.dt.bfloat16
    FP8 = mybir.dt.float8e4
    I16 = mybir.dt.int16
    Exp = mybir.ActivationFunctionType.Exp
    DR = mybir.MatmulPerfMode.DoubleRow
    Alu = mybir.AluOpType

    nc = bacc.Bacc()

    xT0_d = nc.dram_tensor("xT0", [128, NMAX], F32, kind="ExternalInput")
    xT1_d = nc.dram_tensor("xT1", [128, NMAX], F32, kind="ExternalInput")
    onesf_d = nc.dram_tensor("onesf", [1, NMAX], F32, kind="ExternalInput")
    wqk0_d = nc.dram_tensor("wqk0", [128, 512], F32, kind="ExternalInput")
    wqk1_d = nc.dram_tensor("wqk1", [128, 512], F32, kind="ExternalInput")
    wqkb_d = nc.dram_tensor("wqkb", [1, 512], F32, kind="ExternalInput")
    wv0_d = nc.dram_tensor("wv0", [128, 264], F32, kind="ExternalInput")
    wv1_d = nc.dram_tensor("wv1", [128, 264], F32, kind="ExternalInput")
    wvb_d = nc.dram_tensor("wvb", [1, 264], F32, kind="ExternalInput")
    vmask_d = nc.dram_tensor("vmask", [128, 8], F32, kind="ExternalInput")
    wp8_d = nc.dram_tensor("wp8", [128, 512], BF16, kind="ExternalInput")
    wpb8_d = nc.dram_tensor("wpb8", [1, 256], BF16, kind="ExternalInput")
    ones8_d = nc.dram_tensor("ones8", [1, NMAX], BF16, kind="ExternalInput")
    ident_d = nc.dram_tensor("ident", [128, 128], BF16, kind="ExternalInput")
    out_d = nc.dram_tensor("out", [NMAX, C], F32, kind="ExternalOutput")

    with tile.TileContext(nc) as tc:
        with (
            nc.allow_low_precision("fp8/bf16 attention pipeline; verified vs reference"),
            tc.tile_pool(name="const", bufs=1) as cpool,
            tc.tile_pool(name="qk8", bufs=1) as qk8pool,
            tc.tile_pool(name="vp", bufs=1) as vpool,
            tc.tile_pool(name="pt", bufs=3) as ppool,
            tc.tile_pool(name="oz", bufs=1) as ozpool,
            tc.tile_pool(name="io", bufs=2) as iopool,
            tc.tile_pool(name="ps_s", bufs=3, space="PSUM") as ps_s,
            tc.tile_pool(name="ps_po", bufs=2, space="PSUM") as ps_po,
        ):
            # ---- SBUF constants / inputs ----
            xT0 = cpool.tile([128, NMAX], F32R)
            xT1 = cpool.tile([128, NMAX], F32R)
            onesf = cpool.tile([1, NMAX], F32R)
            wqk0 = cpool.tile([128, 512], F32R)
            wqk1 = cpool.tile([128, 512], F32R)
            wqkb = cpool.tile([1, 512], F32R)
            wv0 = cpool.tile([128, 264], F32R)
            wv1 = cpool.tile([128, 264], F32R)
            wvb = cpool.tile([1, 264], F32R)
            vmask = cpool.tile([128, 8], F32)
            wp8 = cpool.tile([128, 512], BF16)
            wpb8 = cpool.tile([1, 256], BF16)
            ones8 = cpool.tile([1, NMAX], BF16)
            ident = cpool.tile([128, 128], BF16)

            # fp8 Q/K in DoubleRow layout [128 (4h x 32c), 2 sub-tiles, NMAX];
            # sub-tile 1 stays zero.
            q8 = [qk8pool.tile([128, 2, NMAX], FP8, name=f"q8g{g}") for g in range(2)]
            k8 = [qk8pool.tile([128, 2, NMAX], FP8, name=f"k8g{g}") for g in range(2)]

            v_sb = [vpool.tile([128, 264], BF16, name=f"v{i}") for i in range(8)]
            osc = ozpool.tile([128, 8, 256], BF16)       # [q, qc, c] normalized O
            oT = ozpool.tile([128, 2, NMAX], BF16)       # [c-in half, q]

            # DMAs: sync(SP) takes the QK critical path, scalar helps early,
            # gpsimd takes the rest.
            nc.sync.dma_start(wqk0[:], wqk0_d[:].bitcast(F32R))
            nc.sync.dma_start(wqk1[:], wqk1_d[:].bitcast(F32R))
            nc.scalar.dma_start(xT0[:, 0:512], xT0_d[:, 0:512].bitcast(F32R))
            nc.sync.dma_start(xT1[:, 0:512], xT1_d[:, 0:512].bitcast(F32R))
            nc.scalar.dma_start(xT0[:, 512:1024], xT0_d[:, 512:1024].bitcast(F32R))
            nc.sync.dma_start(xT1[:, 512:1024], xT1_d[:, 512:1024].bitcast(F32R))
            nc.gpsimd.dma_start(wqkb[:], wqkb_d[:].bitcast(F32R))
            nc.gpsimd.dma_start(onesf[:], onesf_d[:].bitcast(F32R))
            nc.gpsimd.dma_start(wv0[:], wv0_d[:].bitcast(F32R))
            nc.gpsimd.dma_start(wv1[:], wv1_d[:].bitcast(F32R))
            nc.gpsimd.dma_start(wvb[:], wvb_d[:].bitcast(F32R))
            nc.gpsimd.dma_start(vmask[:], vmask_d[:])
            nc.gpsimd.dma_start(wp8[:], wp8_d[:])
            nc.gpsimd.dma_start(wpb8[:], wpb8_d[:])
            nc.gpsimd.dma_start(ones8[:], ones8_d[:])
            nc.gpsimd.dma_start(ident[:], ident_d[:])
            for g in range(2):
                nc.scalar.memzero(q8[g][:, 1, :])
                nc.scalar.memzero(k8[g][:, 1, :])

            # warm the ACT exp table while DMAs land, and ramp the PE
            # p-state with dummy matmuls on a zeroed tile.
            warm = cpool.tile([1, 1], F32)
            nc.vector.memset(warm[:], 0.0)
            nc.scalar.activation(warm[:], warm[:], Exp, scale=1.0)
            dwarm = cpool.tile([128, 512], F32)
            nc.vector.memset(dwarm[:], 0.0)
            pwm = ps_po.tile([128, 264], F32, tag="po", name="pwm")
            for i in range(6):
                nc.tensor.matmul(pwm[:], dwarm[:, 0:128].bitcast(F32R),
                                 dwarm[:, 0:264].bitcast(F32R),
                                 start=True, stop=True)

            def emit_qk(dst, cs, on_act=False):
                # dst sub-tile 0 <- (x @ wqk[:, cs] + b) as fp8
                pq = ps_s.tile([128, NMAX], F32, tag="s")
                for j in range(2):
                    js = slice(j * 512, (j + 1) * 512)
                    nc.tensor.matmul(pq[:, js], wqk0[:, cs], xT0[:, js],
                                     start=True, stop=False)
                    nc.tensor.matmul(pq[:, js], wqk1[:, cs], xT1[:, js],
                                     start=False, stop=False)
                    nc.tensor.matmul(pq[:, js], wqkb[:, cs], onesf[:, js],
                                     start=False, stop=True)
                for j in range(2):
                    js = slice(j * 512, (j + 1) * 512)
                    if on_act:
                        nc.scalar.copy(dst[:, 0, js], pq[:, js])
                    else:
                        nc.vector.tensor_copy(dst[:, 0, js], pq[:, js])

            def emit_v(nt):
                pv = ps_po.tile([128, 264], F32, tag="po")
                ns = slice(nt * 128, (nt + 1) * 128)
                nc.tensor.matmul(pv[:], xT0[:, ns], wv0[:], start=True, stop=False)
                nc.tensor.matmul(pv[:], xT1[:, ns], wv1[:], start=False, stop=False)
                nc.tensor.matmul(pv[:], onesf[:, ns], wvb[:], start=False, stop=True)
                # zero out masked (padded) key rows, including the rowsum col
                nc.vector.tensor_scalar(v_sb[nt][:], pv[:],
                                        vmask[:, nt : nt + 1], None, Alu.mult)

            p_tiles = {}

            def emit_s_exp(h, kts):
                g, hh = h // 4, h % 4
                rows = slice(32 * hh, 32 * hh + 32)
                if h in p_tiles:
                    p_h = p_tiles[h]
                else:
                    p_h = [ppool.tile([128, NMAX], BF16, tag=f"p{kt}",
                                      name=f"p_h{h}_{kt}") for kt in range(8)]
                    p_tiles[h] = p_h
                for kt in kts:
                    ss = ps_s.tile([128, NMAX], F32, tag="s")
                    ks = slice(kt * 128, (kt + 1) * 128)
                    for j in range(2):
                        js = slice(j * 512, (j + 1) * 512)
                        nc.tensor.matmul(ss[:, js], k8[g][rows, :, ks],
                                         q8[g][rows, :, js], start=True,
                                         stop=True, perf_mode=DR,
                                         tile_position=(32 * hh, 0))
                    if _EXP_ENG[(h, kt)] == "a":
                        nc.scalar.activation(p_h[kt][:], ss[:], Exp,
                                             bias=0.0, scale=1.0 / A16)
                    else:
                        nc.vector.tensor_scalar(p_h[kt][:].bitcast(I16), ss[:],
                                                B16F, 0.0, Alu.add, Alu.max)

            def emit_av_norm(h, qhalf=None):
                if qhalf in (None, 0):
                    p_tiles[h] = (p_tiles[h],
                                  ps_po.tile([128, 264], F32, tag="po",
                                             name=f"po{h}"))
                p_h, po = p_tiles[h]
                po3 = po.rearrange("p (a b) -> p a b", a=8)
                qcs = range(8) if qhalf is None else range(4 * qhalf, 4 * qhalf + 4)
                for qc in qcs:
                    qs = slice(qc * 128, (qc + 1) * 128)
                    for kt in range(8):
                        nc.tensor.matmul(po3[:, qc, :], p_h[kt][:, qs],
                                         v_sb[kt][:, 33 * h : 33 * h + 33],
                                         start=(kt == 0), stop=(kt == 7))
                hs = slice(0, 8) if qhalf is None else slice(4 * qhalf, 4 * qhalf + 4)
                n = 8 if qhalf is None else 4
                rs = cpool.tile([128, 8], F32, tag="rs", bufs=2,
                                name=f"rs{h}_{qhalf}")
                nc.vector.reciprocal(rs[:, 0:n, None], po3[:, hs, 32:33])
                nc.vector.tensor_tensor(
                    osc[:, hs, 32 * h : 32 * h + 32], po3[:, hs, 0:32],
                    rs[:, 0:n, None].broadcast_to([128, n, 32]), Alu.mult)
                if qhalf in (None, 1):
                    p_tiles.pop(h)

            def emit_tr(half, qcs):
                # transpose osc[:, qc, 128*half:+128] -> oT[:, half, :]
                trp = ps_po.tile([128, 512], BF16, tag="po",
                                 padded_shape=[128, 528], name="trp")
                for i, qc in enumerate(qcs):
                    nc.tensor.transpose(
                        trp[:, 128 * i : 128 * i + 128],
                        osc[:, qc, 128 * half : 128 * half + 128], ident[:])
                dst = oT[:, half, 512 * (qcs[0] // 4) : 512 * (qcs[0] // 4) + 512]
                nc.vector.tensor_copy(dst, trp[:, 0:512])

            def emit_proj(qc):
                pf = ps_s.tile([128, 256], F32, tag="s",
                               padded_shape=[128, 1024], name=f"pf{qc}")
                qs = slice(qc * 128, (qc + 1) * 128)
                nc.tensor.matmul(pf[:], oT[:, 0, qs], wp8[:, 0:256],
                                 start=True, stop=False)
                nc.tensor.matmul(pf[:], oT[:, 1, qs], wp8[:, 256:512],
                                 start=False, stop=False)
                nc.tensor.matmul(pf[:], ones8[:, qs], wpb8[:],
                                 start=False, stop=True)
                fo = iopool.tile([128, 256], F32, tag="fo", name=f"fo{qc}")
                if qc % 2 == 0:
                    nc.scalar.copy(fo[:], pf[:])
                else:
                    nc.vector.tensor_copy(fo[:], pf[:])
                nc.sync.dma_start(out_d[qc * 128 : (qc + 1) * 128, :], fo[:])

            # ---- emission schedule ----
            emit_qk(q8[0], slice(0, 128))
            emit_qk(k8[0], slice(256, 384), on_act=True)
            emit_s_exp(0, (0, 1))
            emit_v(0)
            emit_s_exp(0, (2, 3))
            emit_v(1)
            emit_v(2)
            emit_s_exp(0, (4, 5))
            emit_v(3)
            emit_v(4)
            emit_s_exp(0, (6, 7))
            emit_v(5)
            emit_v(6)
            emit_v(7)
            emit_s_exp(1, (0, 1, 2, 3))
            emit_qk(q8[1], slice(128, 256))
            emit_s_exp(1, (4, 5, 6, 7))
            emit_qk(k8[1], slice(384, 512), on_act=True)
            for h in range(2, H):
                emit_s_exp(h, range(8))
                emit_av_norm(h - 2)
                if h == 6:
                    emit_tr(0, (0, 1, 2, 3))
                if h == 7:
                    emit_tr(0, (4, 5, 6, 7))
            emit_av_norm(6, 0)
            emit_av_norm(6, 1)
            emit_av_norm(7, 0)
            emit_tr(1, (0, 1, 2, 3))
            emit_av_norm(7, 1)
            emit_tr(1, (4, 5, 6, 7))
            for qc in range(8):
                emit_proj(qc)

    nc.finalize()
    return nc


def _prep_shared(qkv_w, qkv_b, proj_w, proj_b):
    import ml_dtypes

    wq = qkv_w[:, 0:C] * ALPHA
    wk = qkv_w[:, C : 2 * C] * (SCALE * ALPHA)
    wv = qkv_w[:, 2 * C : 3 * C]
    bq = qkv_b[0:C] * ALPHA
    bk = qkv_b[C : 2 * C] * (SCALE * ALPHA)
    bv = qkv_b[2 * C : 3 * C]

    wqk = np.concatenate([wq, wk], axis=1).astype(np.float32)
    bqk = np.concatenate([bq, bk]).astype(np.float32)[None, :]

    wv_aug = np.zeros((C + 1, 33 * H), dtype=np.float32)
    for h in range(H):
        wv_aug[0:C, 33 * h : 33 * h + 32] = wv[:, 32 * h : 32 * h + 32]
        wv_aug[C, 33 * h : 33 * h + 32] = bv[32 * h : 32 * h + 32]
        wv_aug[C, 33 * h + 32] = 1.0

    wp8 = np.concatenate([proj_w[0:128], proj_w[128:256]], axis=1)

    return {
        "wqk0": np.ascontiguousarray(wqk[0:128]),
        "wqk1": np.ascontiguousarray(wqk[128:256]),
        "wqkb": np.ascontiguousarray(bqk),
        "wv0": np.ascontiguousarray(wv_aug[0:128]),
        "wv1": np.ascontiguousarray(wv_aug[128:256]),
        "wvb": np.ascontiguousarray(wv_aug[256:257]),
        "wp8": np.ascontiguousarray(wp8.astype(ml_dtypes.bfloat16)),
        "wpb8": np.ascontiguousarray(proj_b[None, :].astype(ml_dtypes.bfloat16)),
        "onesf": np.ones((1, NMAX), dtype=np.float32),
        "ones8": np.ones((1, NMAX), dtype=ml_dtypes.bfloat16),
        "ident": np.eye(128, dtype=np.float32).astype(ml_dtypes.bfloat16),
    }


def _numpy_fallback(data, qkv_w, qkv_b, proj_w, proj_b, ct_mask, batch_id, pos_id):
    x = np.zeros((B, NMAX, C), dtype=np.float32)
    x[batch_id, pos_id] = data
    qkv = (x @ qkv_w + qkv_b).reshape(B, NMAX, 3, H, HD)
    q = np.moveaxis(qkv[:, :, 0], 2, 1)
    k = np.moveaxis(qkv[:, :, 1], 2, 1)
    v = np.moveaxis(qkv[:, :, 2], 2, 1)
    attn = np.einsum("bhqd,bhkd->bhqk", q * SCALE, k) + ct_mask[:, None]
    attn = attn - attn.max(axis=-1, keepdims=True)
    attn = np.exp(attn)
    attn /= attn.sum(axis=-1, keepdims=True)
    out = np.einsum("bhqk,bhkd->bhqd", attn, v)
    out = np.moveaxis(out, 1, 2).reshape(B, NMAX, C)
    out = out[batch_id, pos_id]
    return (out @ proj_w + proj_b).astype(np.float32)


def kernel(data, qkv_w, qkv_b, proj_w, proj_b, ct_mask, batch_id, pos_id,
           _profile=False):
    from concourse.bass_utils import run_bass_kernel_spmd

    data = np.asarray(data, dtype=np.float32)
    qkv_w = np.asarray(qkv_w, dtype=np.float32)
    qkv_b = np.asarray(qkv_b, dtype=np.float32)
    proj_w = np.asarray(proj_w, dtype=np.float32)
    proj_b = np.asarray(proj_b, dtype=np.float32)
    ct_mask = np.asarray(ct_mask, dtype=np.float32)
    batch_id = np.asarray(batch_id)
    pos_id = np.asarray(pos_id)

    # Device path needs the mask constant along the query axis and binary
    # (0 for valid keys / large-negative for padded keys): padded keys are
    # then dropped by zeroing V rows. Otherwise fall back.
    mask_vec = ct_mask[:, 0, :]
    binary = np.all((mask_vec == 0.0) | (mask_vec <= -1e3))
    if not binary or not np.array_equal(
        ct_mask, np.broadcast_to(mask_vec[:, None, :], ct_mask.shape)
    ):
        return _numpy_fallback(data, qkv_w, qkv_b, proj_w, proj_b, ct_mask,
                               batch_id, pos_id)

    x = np.zeros((B, NMAX, C), dtype=np.float32)
    x[batch_id, pos_id] = data

    shared = _prep_shared(qkv_w, qkv_b, proj_w, proj_b)

    if "nc" not in _CACHE:
        _CACHE["nc"] = _build_program()
    nc = _CACHE["nc"]

    valid = (mask_vec == 0.0).astype(np.float32)  # [B, NMAX]

    in_maps = []
    for b in range(B):
        xT = np.ascontiguousarray(x[b].T)
        im = dict(shared)
        im["xT0"] = xT[0:128]
        im["xT1"] = xT[128:256]
        im["vmask"] = np.ascontiguousarray(valid[b].reshape(8, 128).T)
        in_maps.append(im)

    res = run_bass_kernel_spmd(nc, in_maps, core_ids=list(range(B)))
    if _profile:
        _CACHE["last_results"] = res

    out_pad = np.stack([res.results[b]["out"] for b in range(B)])
    return out_pad[batch_id, pos_id].astype(np.float32)
